# revision 1
# baseline (speedup 1.0000x reference)
# Trainium2 Bass kernel for nn_AttentionLayer_69380901699611.
#
# Full-input contract: kernel(**inputs) takes the unsharded numpy inputs and
# returns the full [B, F, HIDDEN] output. Internally the work is sharded over
# 8 NeuronCores as (batch x head-group): core c handles batch c//4 and heads
# [4*(c%4), 4*(c%4)+4). Each core computes a partial output projection over
# its 4 heads; the host sums the 4 partials per batch.
#
# Device kernel layout (per core):
#   qT, kT      [NH=256 part-chunks, F/T] bf16 (head-dim on partitions)
#   v           [T part, NH] bf16 with an appended ones column per head (the
#               softmax denominator falls out of the attn matmul for free)
#   scores^T    [T part, F free] fp32 psum = kT_chunk.T @ qT_chunk (K=64 pairs
#               on row groups 0-63 / 64-127)
#   softmax     exp on ACT (psum -> bf16), then multiply by exp(bias)^T
#               (precomputed on host, bf16) on DVE in 2x bf16 mode.
#               No max-subtraction needed: |logits| <~ 12.
#   attn^T      [H+1, F] accumulated over T tiles in PSUM (fp32)
#   out         attn^T.T @ wo accumulated over the two 128-row NH chunks; the
#               output projection for each F chunk is emitted right after the
#               chunk's normalize so it overlaps the next chunk's main loop.

import numpy as np

B, F, T, C = 2, 2048, 2048, 1024
HEADS, DEPTH = 16, 64
N_CORES = 8
HG = 4  # head-groups; heads per group = HEADS // HG = 4


def build_attention_nc(C=1024, F=2048, T=2048, NHEADS=4, H=64, fc_w=512,
                       debug_taps=False):
    import concourse.tile as tile
    import concourse.mybir as mybir
    from concourse import bacc

    P = 128
    NH = NHEADS * H          # local heads * depth (256)
    KC = C // P              # contraction subtiles for projections (8)
    NFC = F // fc_w          # F chunks (4)
    NTT = T // P             # T tiles (16)
    NHC = NH // P            # NH chunks of 128 partitions (2)
    FPC = fc_w // P          # F tiles per F chunk (4)
    assert NHC * 2 == NHEADS and H == 64, "layout assumes 2 heads per NH chunk"
    f32 = mybir.dt.float32
    bf16 = mybir.dt.bfloat16
    scale = float(H) ** -0.5
    Exp = mybir.ActivationFunctionType.Exp
    Mult = mybir.AluOpType.mult

    nc = bacc.Bacc("TRN2", target_bir_lowering=False, debug=False, name="attn69")

    qT_d = nc.dram_tensor("qT", [C, F], bf16, kind="ExternalInput")
    sT_d = nc.dram_tensor("sT", [C, T], bf16, kind="ExternalInput")
    eb_d = nc.dram_tensor("ebT", [T, F], bf16, kind="ExternalInput")
    wq_d = nc.dram_tensor("wq", [C, NH], bf16, kind="ExternalInput")
    wk_d = nc.dram_tensor("wk", [C, NH], bf16, kind="ExternalInput")
    wv_d = nc.dram_tensor("wv", [C, NH], bf16, kind="ExternalInput")
    wo_d = nc.dram_tensor("wo", [NH, C], bf16, kind="ExternalInput")
    out_d = nc.dram_tensor("out_p", [F, C], f32, kind="ExternalOutput")

    with tile.TileContext(nc) as tc:
        with (
            tc.tile_pool(name="constp", bufs=1) as constp,
            tc.tile_pool(name="persist", bufs=1) as persist,
            tc.tile_pool(name="actp", bufs=4) as actp,
            tc.tile_pool(name="biasp", bufs=6) as biasp,
            tc.tile_pool(name="ptp", bufs=3) as ptp,
            tc.tile_pool(name="smallp", bufs=4) as smallp,
            tc.tile_pool(name="outp", bufs=6) as outp,
            tc.tile_pool(name="psA", bufs=4, space="PSUM") as psA,
            tc.tile_pool(name="psS", bufs=2, space="PSUM") as psS,
        ):
            # ---------------- weights ----------------
            wq_sb = constp.tile([P, KC, NH], bf16, name="wq_sb")
            nc.sync.dma_start(wq_sb[:], wq_d.ap().rearrange("(ko p) m -> p ko m", p=P))
            wk_sb = constp.tile([P, KC, NH], bf16, name="wk_sb")
            nc.sync.dma_start(wk_sb[:], wk_d.ap().rearrange("(ko p) m -> p ko m", p=P))
            wv_sb = constp.tile([P, KC, NH], bf16, name="wv_sb")
            nc.sync.dma_start(wv_sb[:], wv_d.ap().rearrange("(ko p) m -> p ko m", p=P))
            wo_sb = constp.tile([P, NHC, C], bf16, name="wo_sb")
            nc.sync.dma_start(wo_sb[:], wo_d.ap().rearrange("(ko p) m -> p ko m", p=P))

            # ---------------- persistent activations ----------------
            qT_sb = persist.tile([P, NHC, F], bf16, name="qT_sb")
            kT_sb = persist.tile([P, NHC, T], bf16, name="kT_sb")
            v_sb = persist.tile([P, NTT, NHEADS, H + 1], bf16, name="v_sb")
            attn_sb = persist.tile([P, NHC, F], bf16, name="attn_sb")
            # ones column for the softmax denominator (cols 0..H-1 overwritten)
            ones1 = nc.const_aps.aps[(f32, 1.0)]
            nc.scalar.copy(
                v_sb[:],
                ones1[:, None, None, :].to_broadcast((P, NTT, NHEADS, H + 1)))

            # ---------------- q projection (emitted per F chunk) ----------------
            qT_r = qT_d.ap().rearrange("(ko p) f -> p ko f", p=P)

            def q_proj(fc):
                qa = actp.tile([P, KC, fc_w], bf16, tag="act", name="qa")
                nc.sync.dma_start(qa[:], qT_r[:, :, fc * fc_w:(fc + 1) * fc_w])
                for m in range(NHC):
                    psq = psA.tile([P, 512], f32, tag="bank", name="psq")
                    for k in range(KC):
                        nc.tensor.matmul(
                            psq[:, :fc_w],
                            lhsT=wq_sb[:, k, m * P:(m + 1) * P],
                            rhs=qa[:, k, :],
                            start=(k == 0), stop=(k == KC - 1))
                    # q * depth**-0.5, psum fp32 -> bf16 (on ACT)
                    nc.scalar.mul(
                        qT_sb[:, m, fc * fc_w:(fc + 1) * fc_w], psq[:, :fc_w], scale)

            # ---------------- k and v projections ----------------
            sT_r = sT_d.ap().rearrange("(ko p) t -> p ko t", p=P)
            t_per_chunk = fc_w // P

            def kv_proj(sc):
                sa = actp.tile([P, KC, fc_w], bf16, tag="act", name="sa")
                nc.sync.dma_start(sa[:], sT_r[:, :, sc * fc_w:(sc + 1) * fc_w])
                for m in range(NHC):
                    psk = psA.tile([P, 512], f32, tag="bank", name="psk")
                    for k in range(KC):
                        nc.tensor.matmul(
                            psk[:, :fc_w],
                            lhsT=wk_sb[:, k, m * P:(m + 1) * P],
                            rhs=sa[:, k, :],
                            start=(k == 0), stop=(k == KC - 1))
                    nc.vector.tensor_copy(kT_sb[:, m, sc * fc_w:(sc + 1) * fc_w], psk[:, :fc_w])
                for tl in range(t_per_chunk):
                    tt = sc * t_per_chunk + tl
                    psv = psA.tile([P, 512], f32, tag="bank", name="psv")
                    for k in range(KC):
                        nc.tensor.matmul(
                            psv[:, :NH],
                            lhsT=sa[:, k, tl * P:(tl + 1) * P],
                            rhs=wv_sb[:, k, :],
                            start=(k == 0), stop=(k == KC - 1))
                    nc.vector.tensor_copy(
                        v_sb[:, tt, :, 0:H],
                        psv[:, :NH].rearrange("p (h x) -> p h x", h=NHEADS))

            # ------------- attention main loop (software-pipelined) -------------
            # Chunk fc's softmax stream (ST matmuls -> exp -> *exp(bias))
            # produces NTT pt tiles; chunk fc-1's attention accumulation,
            # normalize, and output projection are interleaved with it. Chunk
            # 0's stream overlaps the k/v projection prefix, so ACT/DVE are
            # busy during the PE-dense projection phase and across chunk
            # boundaries.
            pt_store = {}

            def produce(fc, tt):
                fsl = slice(fc * fc_w, (fc + 1) * fc_w)
                bias_t = biasp.tile([P, fc_w], bf16, tag="bias", name="bias_t")
                nc.sync.dma_start(bias_t[:], eb_d.ap()[tt * P:(tt + 1) * P, fsl])
                pt4 = ptp.tile([P, NHEADS, fc_w], bf16, tag="pt", bufs=20, name="pt4")
                for pair in range(NHC):
                    st2 = psS.tile([P, 2, 512], f32, tag="st", name="st2")
                    for j in range(2):
                        off = j * H
                        nc.tensor.matmul(
                            st2[:, j, :fc_w],
                            lhsT=kT_sb[off:off + H, pair, tt * P:(tt + 1) * P],
                            rhs=qT_sb[off:off + H, pair, fsl],
                            start=True, stop=True)
                    # exp(S^T), psum fp32 -> bf16
                    nc.scalar.activation(
                        pt4[:, 2 * pair:2 * pair + 2, :], st2[:, :, :fc_w], Exp)
                # multiply by exp(bias)^T tile, bf16 2x mode; bias is
                # broadcast over the head dim (stride-0) to halve instr count
                for g in range(NHEADS // 2):
                    nc.vector.tensor_mul(
                        pt4[:, 2 * g:2 * g + 2, :], pt4[:, 2 * g:2 * g + 2, :],
                        bias_t[:, None, :].to_broadcast((P, 2, fc_w)))
                pt_store[(fc, tt)] = pt4

            def consume(fc, tt, at_tiles):
                pt4 = pt_store.pop((fc, tt))
                for h in range(NHEADS):
                    nc.tensor.matmul(
                        at_tiles[h][0:H + 1, :fc_w],
                        lhsT=v_sb[:, tt, h, :],
                        rhs=pt4[:, h, :],
                        start=(tt == 0), stop=(tt == NTT - 1))

            def finish(fc, at_tiles):
                # normalize: attn /= denominator (row H of each at tile)
                fsl = slice(fc * fc_w, (fc + 1) * fc_w)
                for h in range(NHEADS):
                    pair, j = h // 2, h % 2
                    rec = smallp.tile([H + 1, fc_w], f32, tag="r1", name="rec")
                    nc.vector.reciprocal(rec[H:H + 1, :], at_tiles[h][H:H + 1, :fc_w])
                    # partition_broadcast reads partition 0 of its source tile
                    # regardless of AP base: DMA the row down to partition 0.
                    rec0 = smallp.tile([1, fc_w], f32, tag="r0", name="rec0")
                    nc.sync.dma_start(rec0[:], rec[H:H + 1, :])
                    rec64 = smallp.tile([H, fc_w], f32, tag="r64", name="rec64")
                    nc.gpsimd.partition_broadcast(rec64[:], rec0[:])
                    if j == 0:
                        nc.vector.tensor_tensor(
                            attn_sb[0:H, pair, fsl],
                            at_tiles[h][0:H, :fc_w], rec64[:], Mult)
                    else:
                        stg = smallp.tile([H, fc_w], bf16, tag="stg", name="stg")
                        nc.vector.tensor_tensor(
                            stg[:], at_tiles[h][0:H, :fc_w], rec64[:], Mult)
                        nc.sync.dma_start(attn_sb[H:2 * H, pair, fsl], stg[:])
                # output projection for this F chunk
                cw = min(512, C)
                for fl in range(FPC):
                    ft = fc * FPC + fl
                    for cc in range(C // cw):
                        pso = psA.tile([P, 512], f32, tag="bank", name="pso")
                        for m in range(NHC):
                            nc.tensor.matmul(
                                pso[:, :cw],
                                lhsT=attn_sb[:, m, ft * P:(ft + 1) * P],
                                rhs=wo_sb[:, m, cc * cw:(cc + 1) * cw],
                                start=(m == 0), stop=(m == NHC - 1))
                        ot = outp.tile([P, 512], f32, tag="o", name="ot")
                        nc.vector.tensor_copy(ot[:, :cw], pso[:, :cw])
                        nc.sync.dma_start(
                            out_d.ap()[ft * P:(ft + 1) * P, cc * cw:(cc + 1) * cw],
                            ot[:, :cw])

            # chunk 0 production rides along with the k/v projections above;
            # emit it interleaved per source chunk was not possible there, so
            # do it now (the scheduler still overlaps via dataflow), then
            # pipeline chunks 1..NFC-1 against consumption of the previous.
            # emission order: q0, then per source chunk [k/v projections +
            # chunk-0 softmax stream], then q1..3, then the pipelined chunks.
            q_proj(0)
            for sc in range(T // fc_w):
                kv_proj(sc)
                for tl in range(t_per_chunk):
                    produce(0, sc * t_per_chunk + tl)
            for fc in range(1, NFC):
                q_proj(fc)

            def alloc_at():
                return [
                    psA.tile([P, 512], f32, tag="bank", name=f"at{h}")
                    for h in range(NHEADS)
                ]

            LAG = NTT // 2
            at_last = None
            for fc in range(1, NFC + 1):
                if fc == NFC - 1:
                    # Last produced chunk: drain chunk fc-1 at double pace,
                    # then begin consuming chunk fc with a half-chunk lag so
                    # the un-overlappable tail halves.
                    at_prev = alloc_at()
                    for tt in range(NTT):
                        produce(fc, tt)
                        if tt < LAG:
                            consume(fc - 1, 2 * tt, at_prev)
                            consume(fc - 1, 2 * tt + 1, at_prev)
                        else:
                            if tt == LAG:
                                finish(fc - 1, at_prev)
                                at_last = alloc_at()
                            consume(fc, tt - LAG, at_last)
                elif fc == NFC:
                    for tt in range(LAG, NTT):
                        consume(fc - 1, tt, at_last)
                    finish(fc - 1, at_last)
                else:
                    at_prev = alloc_at()
                    for tt in range(NTT):
                        produce(fc, tt)
                        consume(fc - 1, tt, at_prev)
                    finish(fc - 1, at_prev)


    nc.compile()
    return nc


_CACHE = {}


def _get_nc():
    if "nc" not in _CACHE:
        _CACHE["nc"] = build_attention_nc(C=C, F=F, T=T, NHEADS=HEADS // HG, H=DEPTH)
    return _CACHE["nc"]


def kernel(query_input, source_input, bias, wq, wk, wv, wo, **run_kwargs):
    import ml_dtypes
    from concourse.bass_utils import run_bass_kernel_spmd

    bf = ml_dtypes.bfloat16
    q = np.asarray(query_input, dtype=np.float32)
    s = np.asarray(source_input, dtype=np.float32)
    b = np.asarray(bias, dtype=np.float32)
    wq2 = np.asarray(wq, dtype=np.float32).reshape(C, HEADS * DEPTH)
    wk2 = np.asarray(wk, dtype=np.float32).reshape(C, HEADS * DEPTH)
    wv2 = np.asarray(wv, dtype=np.float32).reshape(C, HEADS * DEPTH)
    wo2 = np.asarray(wo, dtype=np.float32).reshape(HEADS * DEPTH, C)

    qT = [np.ascontiguousarray(q[i].T).astype(bf) for i in range(B)]
    sT = [np.ascontiguousarray(s[i].T).astype(bf) for i in range(B)]
    ebT = np.exp(np.ascontiguousarray(b[0, 0].T)).astype(bf)

    nhl = (HEADS // HG) * DEPTH  # NH columns per core (256)
    in_maps = []
    for c in range(N_CORES):
        bi, hg = c // HG, c % HG
        sl = slice(hg * nhl, (hg + 1) * nhl)
        in_maps.append({
            "qT": qT[bi],
            "sT": sT[bi],
            "ebT": ebT,
            "wq": np.ascontiguousarray(wq2[:, sl]).astype(bf),
            "wk": np.ascontiguousarray(wk2[:, sl]).astype(bf),
            "wv": np.ascontiguousarray(wv2[:, sl]).astype(bf),
            "wo": np.ascontiguousarray(wo2[sl, :]).astype(bf),
        })

    nc = _get_nc()
    res = run_bass_kernel_spmd(nc, in_maps, core_ids=list(range(N_CORES)), **run_kwargs)
    _CACHE["last_results"] = res

    out = np.empty((B, F, C), np.float32)
    for bi in range(B):
        acc = res.results[bi * HG]["out_p"].astype(np.float32)
        for hg in range(1, HG):
            acc = acc + res.results[bi * HG + hg]["out_p"]
        out[bi] = acc
    return out


# ---------------------------------------------------------------------------
# Performance state (cost-model timeline, per core): 225.8 us.
# Progression: 331 -> 259 -> 247 -> 244 -> 238 -> 232 -> 226 us, each step
# validated on the 8 NeuronCores at relative error 4.05e-3.
#
# Residual bottlenecks and next steps, in priority order:
# - Main loop is ACT-bound: 128 exp instructions of N=1024 at (N+352)/1.2 ns.
#   N is capped by PSUM: 8 banks = 2 score-tile slots (2 banks each) + 4
#   attn accumulators. Any scheme freeing 2 banks would allow N=2048 exps
#   (-25 us ACT) but single-buffered score tiles serialize PE against ACT
#   (measured worse); needs a restructure of the accumulator layout.
# - The K=64 score matmuls are emitted as concurrent row-group pairs
#   (tile_position (0,0)/(64,0)). The serial cost model charges ~55 us; real
#   silicon overlaps pairs (docs: 4x K=32 row tiles -> 3.07x), so expect
#   ~-25 us PE on hardware vs the model.
# - Projection prefix (~55 us) is PE-saturated with full-rate K=128 matmuls;
#   irreducible without changing math.
# - attn matmuls run M=65 of 128 partitions; N-cycle-bound, so the idle
#   partitions cost nothing in time - no win available (moving operand
#   differs per head, so M-packing is impossible).
# ---------------------------------------------------------------------------



# revision 8
# speedup vs baseline: 1.2108x; 1.2108x over previous
# Trainium2 Bass kernel for nn_AttentionLayer_69380901699611.
#
# Full-input contract: kernel(**inputs) takes the unsharded numpy inputs and
# returns the full [B, F, HIDDEN] output. Internally the work is sharded over
# 8 NeuronCores as (batch x head-group): core c handles batch c//4 and heads
# [4*(c%4), 4*(c%4)+4). Each core computes a partial output projection over
# its 4 heads; the host sums the 4 partials per batch.
#
# v2 design (ACT-bound):
#   - q/k/v projections and the score matmuls run in fp8-e4m3 with the
#     DoubleRow perf mode (0.5 PE cycles/row, 2 k-tiles per instruction).
#     Weights are scaled x32 on the host so e4m3 has mantissa to work with;
#     the scale is undone by the exp's scale immediate (2^-13 = 1/(32*32*8),
#     folding in the 1/sqrt(depth) factor) and by wo/32.
#   - For scores, head h's 64-deep contraction is split into 2 k-tiles of 32
#     held at SBUF partitions 32h..32h+31 (host reorders weight columns into
#     A=d0-31 / B=d32-63 blocks), so one DR matmul per (head, T-tile) emits
#     [T=128, F=512] scores.
#   - softmax: exp on ACT (PSUM->SBUF bf16, scale immediate), then *exp(bias)
#     with DVE (heads 0-2, one 2x-mode instr) + GPSIMD (head 3) so DVE stays
#     under the ACT roofline. No max-subtraction needed: |logits/8| <~ 12.
#   - attention accumulates in [F-part 128, head, 65] layout (full-partition
#     matmuls with pt as stationary, v+ones as moving; col 64 = softmax
#     denominator), one PSUM bank per 128-F tile. Normalize is then a tiny
#     per-F-tile reciprocal+mult, PE-transposed into attnT for the bf16
#     output projection.
#   - ACT runs 128 exp instructions of N=1024 back-to-back (~133us); all
#     other engines are strictly below that.

import numpy as np

B, F, T, C = 2, 2048, 2048, 1024
HEADS, DEPTH = 16, 64
N_CORES = 8
HG = 4   # head-groups; heads per group = HEADS // HG = 4
WSC = 32.0  # host-side weight scale for e4m3


def build_attention_nc(C=1024, F=2048, T=2048, NHEADS=4, H=64, fc_w=512):
    import concourse.tile as tile
    import concourse.mybir as mybir
    from concourse import bacc

    P = 128
    NH = NHEADS * H          # 256
    KC = C // P              # 8 k-tiles for the projections
    NFC = F // fc_w          # 4 F chunks
    NTT = T // P             # 16 T tiles
    FPC = fc_w // P          # 4 F tiles per chunk
    f32 = mybir.dt.float32
    bf16 = mybir.dt.bfloat16
    f8e4 = mybir.dt.float8e4
    Exp = mybir.ActivationFunctionType.Exp
    Mult = mybir.AluOpType.mult
    DR = mybir.MatmulPerfMode.DoubleRow
    ESCALE = 1.0 / (WSC * WSC * H ** 0.5)  # 2^-13

    nc = bacc.Bacc("TRN2", target_bir_lowering=False, debug=False, name="attn69")

    qT_d = nc.dram_tensor("qT", [C, F], bf16, kind="ExternalInput")
    sT_d = nc.dram_tensor("sT", [C, T], bf16, kind="ExternalInput")
    eb_d = nc.dram_tensor("ebT", [T, F], bf16, kind="ExternalInput")
    # wq/wk columns: [A: h0 d0-31 | h1 d0-31 | ... | B: h0 d32-63 | ...]
    wq_d = nc.dram_tensor("wq", [C, 2, P], bf16, kind="ExternalInput")
    wk_d = nc.dram_tensor("wk", [C, 2, P], bf16, kind="ExternalInput")
    wv_d = nc.dram_tensor("wv", [C, NH], bf16, kind="ExternalInput")
    wo_d = nc.dram_tensor("wo", [NH, C], bf16, kind="ExternalInput")
    id_d = nc.dram_tensor("ident", [P, P], f32, kind="ExternalInput")
    out_d = nc.dram_tensor("out_p", [F, C], f32, kind="ExternalOutput")

    with tile.TileContext(nc) as tc:
        with (
            tc.tile_pool(name="constp", bufs=1) as constp,
            tc.tile_pool(name="persist", bufs=1) as persist,
            tc.tile_pool(name="actp", bufs=4) as actp,
            tc.tile_pool(name="biasp", bufs=6) as biasp,
            tc.tile_pool(name="ptp", bufs=20) as ptp,
            tc.tile_pool(name="flatp", bufs=2) as flatp,
            tc.tile_pool(name="smallp", bufs=4) as smallp,
            tc.tile_pool(name="outp", bufs=6) as outp,
            tc.tile_pool(name="psA", bufs=4, space="PSUM") as psA,
            tc.tile_pool(name="psS", bufs=2, space="PSUM") as psS,
        ):
            # ---------------- weights ----------------
            wq_sb = constp.tile([P, KC, 2, P], bf16, name="wq_sb")
            nc.sync.dma_start(wq_sb[:], wq_d.ap().rearrange("(ko p) a m -> p ko a m", p=P))
            wk_sb = constp.tile([P, KC, 2, P], bf16, name="wk_sb")
            nc.sync.dma_start(wk_sb[:], wk_d.ap().rearrange("(ko p) a m -> p ko a m", p=P))
            wv_sb = constp.tile([P, KC, NH], bf16, name="wv_sb")
            nc.sync.dma_start(wv_sb[:], wv_d.ap().rearrange("(ko p) m -> p ko m", p=P))
            wo_sb = constp.tile([P, 2, C], bf16, name="wo_sb")
            nc.sync.dma_start(wo_sb[:], wo_d.ap().rearrange("(ko p) m -> p ko m", p=P))
            ident = constp.tile([P, P], f32, name="ident")
            nc.sync.dma_start(ident[:], id_d.ap())

            # ---------------- persistent activations ----------------
            # qT/kT: [4 heads x 32 depth on partitions, A/B k-tile, cols]
            qT_sb = persist.tile([P, 2, F], f8e4, name="qT_sb")
            kT_sb = persist.tile([P, 2, T], f8e4, name="kT_sb")
            v_sb = persist.tile([P, NTT, NHEADS, H + 1], bf16, name="v_sb")
            attnT_sb = persist.tile([P, 2, F], bf16, name="attnT_sb")
            # ones column for the softmax denominator
            nc.vector.memset(v_sb[:, :, :, H:H + 1], 1.0)

            qT_r = qT_d.ap().rearrange("(ko p) f -> p ko f", p=P)
            sT_r = sT_d.ap().rearrange("(ko p) t -> p ko t", p=P)

            def q_proj(fc):
                fsl = slice(fc * fc_w, (fc + 1) * fc_w)
                qa = actp.tile([P, KC, fc_w], bf16, tag="act", name="qa")
                nc.sync.dma_start(qa[:], qT_r[:, :, fsl])
                for a in range(2):
                    psq = psA.tile([P, 512], f32, tag="bank", name="psq")
                    for k in range(KC):
                        nc.tensor.matmul(
                            psq[:, :fc_w],
                            lhsT=wq_sb[:, k, a, :],
                            rhs=qa[:, k, :],
                            start=(k == 0), stop=(k == KC - 1))
                    nc.vector.tensor_copy(qT_sb[:, a, fsl], psq[:, :fc_w])

            def kv_proj(sc):
                ssl = slice(sc * fc_w, (sc + 1) * fc_w)
                sa = actp.tile([P, KC, fc_w], bf16, tag="act", name="sa")
                nc.sync.dma_start(sa[:], sT_r[:, :, ssl])
                for a in range(2):
                    psk = psA.tile([P, 512], f32, tag="bank", name="psk")
                    for k in range(KC):
                        nc.tensor.matmul(
                            psk[:, :fc_w],
                            lhsT=wk_sb[:, k, a, :],
                            rhs=sa[:, k, :],
                            start=(k == 0), stop=(k == KC - 1))
                    nc.vector.tensor_copy(kT_sb[:, a, ssl], psk[:, :fc_w])
                for tl in range(fc_w // P):
                    tt = sc * (fc_w // P) + tl
                    psv = psA.tile([P, 512], f32, tag="bank", name="psv")
                    for k in range(KC):
                        nc.tensor.matmul(
                            psv[:, :NH],
                            lhsT=sa[:, k, tl * P:(tl + 1) * P],
                            rhs=wv_sb[:, k, :],
                            start=(k == 0), stop=(k == KC - 1))
                    nc.vector.tensor_copy(
                        v_sb[:, tt, :, 0:H],
                        psv[:, :NH].rearrange("p (h x) -> p h x", h=NHEADS))

            # ------------- softmax stream -------------
            pt_store = {}

            def produce(fc, tt):
                fsl = slice(fc * fc_w, (fc + 1) * fc_w)
                tsl = slice(tt * P, (tt + 1) * P)
                bias_t = biasp.tile([P, fc_w], bf16, tag="bias", name="bias_t")
                nc.sync.dma_start(bias_t[:], eb_d.ap()[tsl, fsl])
                pt4 = ptp.tile([P, NHEADS, fc_w], bf16, tag="pt", name="pt4")
                for pair in range(2):
                    st2 = psS.tile([P, 2, 512], f32, tag="st", name="st2")
                    for j in range(2):
                        h = 2 * pair + j
                        nc.tensor.matmul(
                            st2[:, j, :fc_w],
                            lhsT=kT_sb[32 * h:32 * h + 32, :, tsl],
                            rhs=qT_sb[32 * h:32 * h + 32, :, fsl],
                            start=True, stop=True,
                            perf_mode=DR, tile_position=(32 * h, 0))
                    nc.scalar.activation(
                        pt4[:, 2 * pair:2 * pair + 2, :], st2[:, :, :fc_w],
                        Exp, scale=ESCALE)
                # *exp(bias): heads 0-2 on DVE (2x mode), head 3 on GPSIMD
                nc.vector.tensor_mul(
                    pt4[:, 0:3, :], pt4[:, 0:3, :],
                    bias_t[:, None, :].to_broadcast((P, 3, fc_w)))
                nc.gpsimd.tensor_tensor(
                    pt4[:, 3, :], pt4[:, 3, :], bias_t[:], Mult)
                pt_store[(fc, tt)] = pt4

            def alloc_at():
                ats = []
                for fl in range(FPC):
                    raw = psA.tile([P, 512], f32, tag="bank", name=f"at{fl}")
                    ats.append(raw[:, :NHEADS * (H + 1)].rearrange(
                        "p (h x) -> p h x", h=NHEADS))
                return ats

            def consume(fc, tt, ats):
                # One PSUM accumulation group per bank: start=True lazily
                # zeroes the WHOLE 2KB zero region, so only the first write
                # into each F-tile bank may carry it; only the last carries
                # stop (sim-only bookkeeping).
                pt4 = pt_store.pop((fc, tt))
                for fl in range(FPC):
                    for h in range(NHEADS):
                        nc.tensor.matmul(
                            ats[fl][:, h, :],
                            lhsT=pt4[:, h, fl * P:(fl + 1) * P],
                            rhs=v_sb[:, tt, h, :],
                            start=(tt == 0 and h == 0),
                            stop=(tt == NTT - 1 and h == NHEADS - 1))

            def finish(fc, ats):
                fsl = slice(fc * fc_w, (fc + 1) * fc_w)
                flat = flatp.tile([P, FPC, NHEADS, H], f32, tag="flat", name="flat")
                for fl in range(FPC):
                    rec = smallp.tile([P, NHEADS, 1], f32, tag="rec", name="rec")
                    nc.vector.reciprocal(rec[:], ats[fl][:, :, H:H + 1])
                    nc.vector.tensor_tensor(
                        flat[:, fl, :, :], ats[fl][:, :, 0:H],
                        rec.to_broadcast((P, NHEADS, H)), Mult)
                for m in range(2):
                    tr = psA.tile([P, 512], f32, tag="bank", name="tr")
                    for fl in range(FPC):
                        nc.tensor.transpose(
                            tr[:, fl * P:(fl + 1) * P],
                            flat[:, fl, 2 * m:2 * m + 2, :], ident[:])
                    nc.vector.tensor_copy(attnT_sb[:, m, fsl], tr[:])
                # output projection for this F chunk
                for fl in range(FPC):
                    ft = fc * FPC + fl
                    for cc in range(2):
                        pso = psA.tile([P, 512], f32, tag="bank", name="pso")
                        for m in range(2):
                            nc.tensor.matmul(
                                pso[:],
                                lhsT=attnT_sb[:, m, ft * P:(ft + 1) * P],
                                rhs=wo_sb[:, m, cc * 512:(cc + 1) * 512],
                                start=(m == 0), stop=(m == 1))
                        ot = outp.tile([P, 512], f32, tag="o", name="ot")
                        nc.vector.tensor_copy(ot[:], pso[:])
                        nc.sync.dma_start(
                            out_d.ap()[ft * P:(ft + 1) * P, cc * 512:(cc + 1) * 512],
                            ot[:])

            # ------------- schedule -------------
            q_proj(0)
            for sc in range(T // fc_w):
                kv_proj(sc)
                for tl in range(fc_w // P):
                    produce(0, sc * (fc_w // P) + tl)
            for fc in range(1, NFC):
                q_proj(fc)

            for fc in range(1, NFC + 1):
                ats = alloc_at()
                for tt in range(NTT):
                    if fc < NFC:
                        produce(fc, tt)
                    consume(fc - 1, tt, ats)
                finish(fc - 1, ats)

    nc.compile()
    return nc


_CACHE = {}


def _get_nc():
    if "nc" not in _CACHE:
        _CACHE["nc"] = build_attention_nc(C=C, F=F, T=T, NHEADS=HEADS // HG, H=DEPTH)
    return _CACHE["nc"]


def kernel(query_input, source_input, bias, wq, wk, wv, wo, **run_kwargs):
    import ml_dtypes
    from concourse.bass_utils import run_bass_kernel_spmd

    bf = ml_dtypes.bfloat16
    e4 = ml_dtypes.float8_e4m3
    q = np.asarray(query_input, dtype=np.float32)
    s = np.asarray(source_input, dtype=np.float32)
    b = np.asarray(bias, dtype=np.float32)
    wq4 = np.asarray(wq, dtype=np.float32) * WSC   # [C, 16, 64]
    wk4 = np.asarray(wk, dtype=np.float32) * WSC
    wv4 = np.asarray(wv, dtype=np.float32) * WSC
    wo4 = np.asarray(wo, dtype=np.float32) / WSC   # [16, 64, C]

    qT = [np.ascontiguousarray(q[i].T).astype(bf) for i in range(B)]
    sT = [np.ascontiguousarray(s[i].T).astype(bf) for i in range(B)]
    ebT = np.exp(np.ascontiguousarray(b[0, 0].T)).astype(bf)
    ident = np.eye(128, dtype=np.float32)

    nhl = HG  # heads per group
    in_maps = []
    for c in range(N_CORES):
        bi, hg = c // HG, c % HG
        hsl = slice(hg * nhl, (hg + 1) * nhl)
        # wq/wk: [C, 2(A|B), 4 heads, 32 depth] -> [C, 2, 128]
        wqg = wq4[:, hsl, :].reshape(C, HG, 2, 32).transpose(0, 2, 1, 3)
        wkg = wk4[:, hsl, :].reshape(C, HG, 2, 32).transpose(0, 2, 1, 3)
        in_maps.append({
            "qT": qT[bi],
            "sT": sT[bi],
            "ebT": ebT,
            "wq": np.ascontiguousarray(wqg.reshape(C, 2, 128)).astype(bf),
            "wk": np.ascontiguousarray(wkg.reshape(C, 2, 128)).astype(bf),
            "wv": np.ascontiguousarray(
                wv4[:, hsl, :].reshape(C, HG * DEPTH)).astype(bf),
            "wo": np.ascontiguousarray(
                wo4[hsl, :, :].reshape(HG * DEPTH, C)).astype(bf),
            "ident": ident,
        })

    nc = _get_nc()
    res = run_bass_kernel_spmd(nc, in_maps, core_ids=list(range(N_CORES)), **run_kwargs)
    _CACHE["last_results"] = res

    out = np.empty((B, F, C), np.float32)
    for bi in range(B):
        acc = res.results[bi * HG]["out_p"].astype(np.float32)
        for hg in range(1, HG):
            acc = acc + res.results[bi * HG + hg]["out_p"]
        out[bi] = acc
    return out
